# revision 1
# baseline (speedup 1.0000x reference)
"""2-layer GCN (DGL GraphConv norm='both') on 8 Trainium2 NeuronCores.

Strategy (graph/data parallel, dst-sharded):
  - Host: degree norms, pre-scale+transpose features to bf16, sort each
    core's edges by (dst-window, src-half, src), pack into 128-edge tiles.
  - Device per core c (nodes [c*6250, (c+1)*6250)):
      h_sh = XnT_sh.T @ W1            (dense matmuls, bf16, f32 psum)
      AllGather h -> full h table [50176, 128] bf16 in HBM
      agg1 = segment_sum(h[src], dst) via dma_gather (rows->partitions)
             + onehot matmuls accumulating in PSUM per 64-dst window
      x1n = relu((agg1 * ndst + b1) * nsrc)
      AllGather x1n -> full table; same aggregation machinery
      out = W2.T @ (agg2 * ndst).T    per window -> [8, 6250] f32
  - Host: transpose, add b2, concat cores.

Edge tiles: 128 edges, one 64-dst window, one src half (lo: table rows
< 32768, hi: >= 32768 -- dma_gather indices are int16).  The per-window
per-half tile counts are max'd across cores so all 8 cores share one
program; short cores pad with (src=0, localdst=200) no-op edges.
"""

import numpy as np
import ml_dtypes

import concourse.bass as bass
import concourse.bacc as bacc
import concourse.mybir as mybir
import concourse.tile as tile
from concourse import bass_utils

BF16 = ml_dtypes.bfloat16

N = 50000
E = 1600000
FIN = 1433
FP = 1536            # FIN padded to 12*128
H = 128
C = 7
NCORES = 8
NSH = N // NCORES    # 6250
W = 64               # dst window width
NW = (NSH + W - 1) // W          # 98 windows per core
NSHP = NW * W                    # 6272 padded shard rows
NT = N // NCORES + (NSHP - NSH)  # per-core padded rows = 6272
TROWS = NCORES * NSHP            # 50176 table rows
LO = 32768                       # lo/hi table split
BT = 48              # max tiles per dma_gather call
PAD_DST = 200.0      # local-dst sentinel for pad edges (> W-1)

KB = NW * W // 128   # 49 node blocks of 128 in dense stage
KCH = FP // 128      # 12 contraction chunks
DEBUG_TAPS = False   # add dbg_h/dbg_x outputs (copies of gather tables)
GP_BUFS = 3          # gather-tile buffering
OH_BUFS = 2          # onehot-tile buffering
PSW_BUFS = 3         # window psum buffering
NSB = 8              # node blocks per xnt load in dense stage
NSWQ = 1             # SWDGE queues (1 = verified config)


def _ceil_div(a, b):
    return (a + b - 1) // b


def _wrap_idx(idx_flat):
    """[T*128] -> [128, T*8] int16: position i -> [i%16 (+16k copies), i//16]."""
    a = np.asarray(idx_flat, np.int16).reshape(-1, 16).T  # [16, T*8]
    return np.ascontiguousarray(np.tile(a, (8, 1)))       # [128, T*8]


def _prep(features, src, dst, W1, b1, W2, b2):
    """Host-side sharding/packing. Returns (in_maps, schedule, norm info)."""
    src = np.asarray(src).astype(np.int64)
    dst = np.asarray(dst).astype(np.int64)
    features = np.asarray(features, np.float32)

    deg_src = np.bincount(src, minlength=N).astype(np.float32)
    deg_dst = np.bincount(dst, minlength=N).astype(np.float32)
    nsrc = 1.0 / np.sqrt(np.maximum(deg_src, 1.0))
    ndst = 1.0 / np.sqrt(np.maximum(deg_dst, 1.0))

    # padded global table ids
    core_of = src // NSH
    g_src = core_of * NSHP + (src - core_of * NSH)

    # per-core edge groups
    dcore = dst // NSH
    dloc = dst - dcore * NSH
    win = dloc // W
    half = (g_src >= LO).astype(np.int64)

    cnt = np.zeros((NCORES, NW, 2), np.int64)
    per_core = []
    for c in range(NCORES):
        m = dcore == c
        gs, wn, hf, dl = g_src[m], win[m], half[m], dloc[m]
        order = np.lexsort((gs, hf, wn))
        gs, wn, hf, dl = gs[order], wn[order], hf[order], dl[order]
        key = wn * 2 + hf
        cnt[c] = np.bincount(key, minlength=NW * 2).reshape(NW, 2)
        per_core.append((gs, dl, key))

    tw = np.zeros((NW, 2), np.int64)  # shared schedule: tiles per (win, half)
    for h in range(2):
        tw[:, h] = _ceil_div(np.max(cnt[:, :, h], axis=0), 128)

    # tile slots: pass-lo tiles for w=0..NW-1, then pass-hi
    tile_win = []
    tile_base = np.zeros((NW, 2), np.int64)
    for h in range(2):
        for w in range(NW):
            tile_base[w, h] = len(tile_win)
            tile_win.extend([w] * int(tw[w, h]))
    T = len(tile_win)

    # dense-stage feature prep (shared)
    Xn = features * nsrc[:, None]

    w1p = np.zeros((FP, H), np.float32)
    w1p[:FIN] = W1
    w1p = w1p.astype(BF16)
    w2p = np.zeros((H, 8), np.float32)
    w2p[:, :C] = W2
    w2p = w2p.astype(BF16)
    iota = np.tile(np.arange(W, dtype=np.float32), (128, 1)).astype(BF16)
    ident = np.vstack([np.eye(W, dtype=np.float32)] * 2)        # [128, 64]
    identb = ident.astype(BF16)
    b1rep = np.tile(np.asarray(b1, np.float32), (2 * W, 1))     # [128, 128]

    in_maps = []
    for c in range(NCORES):
        gs, dl, key = per_core[c]
        idx_flat = np.zeros(T * 128, np.int64)
        ldst_flat = np.full(T * 128, PAD_DST, np.float32)
        starts = np.zeros(NW * 2 + 1, np.int64)
        starts[1:] = np.cumsum(np.bincount(key, minlength=NW * 2))
        for h in range(2):
            for w in range(NW):
                k = w * 2 + h
                n = starts[k + 1] - starts[k]
                if n == 0:
                    continue
                slot = tile_base[w, h] * 128
                idx_flat[slot:slot + n] = gs[starts[k]:starts[k + 1]] - h * LO
                ldst_flat[slot:slot + n] = dl[starts[k]:starts[k + 1]] % W

        xnt = np.zeros((FP, NSHP), np.float32)
        xnt[:FIN, :NSH] = Xn[c * NSH:(c + 1) * NSH].T

        pad_d = np.zeros(NSHP, np.float32)
        pad_d[:NSH] = ndst[c * NSH:(c + 1) * NSH]
        nsd = pad_d.reshape(NW, W).T
        nsd = np.vstack([nsd, nsd])                             # [128, NW]
        pad_s = np.zeros(NSHP, np.float32)
        pad_s[:NSH] = nsrc[c * NSH:(c + 1) * NSH]
        nss = pad_s.reshape(NW, W).T
        nss = np.vstack([nss, nss])

        in_maps.append({
            "xnt": xnt.astype(BF16),
            "w1": w1p,
            "w2": w2p,
            "iota": iota,
            "identf": ident,
            "identb": identb,
            "b1rep": b1rep,
            "nsd": np.ascontiguousarray(nsd),
            "nss": np.ascontiguousarray(nss),
            "idx": _wrap_idx(idx_flat),
            "ldst": np.ascontiguousarray(
                ldst_flat.reshape(T, 128).T).astype(BF16),
        })
    return in_maps, tw, tile_win, tile_base, T


def _build_program(tw, tile_win, T, timing=False, phases=2):
    nc = bacc.Bacc("TRN2", target_bir_lowering=False, debug=False,
                   num_devices=NCORES, num_swdge_queues=NSWQ)
    dt = mybir.dt
    xnt_d = nc.dram_tensor("xnt", [FP, NSHP], dt.bfloat16, kind="ExternalInput")
    w1_d = nc.dram_tensor("w1", [FP, H], dt.bfloat16, kind="ExternalInput")
    w2_d = nc.dram_tensor("w2", [H, 8], dt.bfloat16, kind="ExternalInput")
    iota_d = nc.dram_tensor("iota", [128, W], dt.bfloat16, kind="ExternalInput")
    identf_d = nc.dram_tensor("identf", [2 * W, W], dt.float32, kind="ExternalInput")
    identb_d = nc.dram_tensor("identb", [2 * W, W], dt.bfloat16, kind="ExternalInput")
    b1_d = nc.dram_tensor("b1rep", [2 * W, H], dt.float32, kind="ExternalInput")
    nsd_d = nc.dram_tensor("nsd", [2 * W, NW], dt.float32, kind="ExternalInput")
    nss_d = nc.dram_tensor("nss", [2 * W, NW], dt.float32, kind="ExternalInput")
    idx_d = nc.dram_tensor("idx", [128, T * 8], dt.int16, kind="ExternalInput")
    ldst_d = nc.dram_tensor("ldst", [128, T], dt.bfloat16, kind="ExternalInput")
    out_d = nc.dram_tensor("out", [8, NSH], dt.float32, kind="ExternalOutput")

    # tile t of pass: window + start/stop flags
    ntl = int(tw[:, 0].sum())

    with tile.TileContext(nc) as tc:
        with (
            tc.tile_pool(name="const", bufs=1) as cpool,
            tc.tile_pool(name="xnt", bufs=2) as xpool,
            tc.tile_pool(name="g", bufs=GP_BUFS) as gpool,
            tc.tile_pool(name="oh", bufs=OH_BUFS) as ohpool,
            tc.tile_pool(name="ep", bufs=2) as eppool,
            tc.tile_pool(name="small", bufs=2) as spool,
            tc.tile_pool(name="psA", bufs=2, space="PSUM") as psA,
            tc.tile_pool(name="psW", bufs=PSW_BUFS, space="PSUM") as psW,
            tc.tile_pool(name="psT", bufs=1, space="PSUM") as psT,
            tc.tile_pool(name="dram", bufs=1, space="DRAM") as dram,
        ):
            # ---- constants ----
            w1_sb = cpool.tile([128, KCH * H], dt.bfloat16, tag="w1")
            nc.sync.dma_start(
                w1_sb[:].rearrange("p (k h) -> p k h", h=H),
                w1_d[:].rearrange("(k p) h -> p k h", p=128))
            w2_sb = cpool.tile([128, 8], dt.bfloat16, tag="w2")
            nc.sync.dma_start(w2_sb[:], w2_d[:])
            iota_sb = cpool.tile([128, W], dt.bfloat16, tag="iota")
            nc.sync.dma_start(iota_sb[:], iota_d[:])
            identf_sb = cpool.tile([2 * W, W], dt.float32, tag="idf")
            nc.sync.dma_start(identf_sb[:], identf_d[:])
            identb_sb = cpool.tile([2 * W, W], dt.bfloat16, tag="idb")
            nc.sync.dma_start(identb_sb[:], identb_d[:])
            b1_sb = cpool.tile([2 * W, H], dt.float32, tag="b1")
            nc.sync.dma_start(b1_sb[:], b1_d[:])
            nsd_sb = cpool.tile([2 * W, NW], dt.float32, tag="nsd")
            nc.sync.dma_start(nsd_sb[:], nsd_d[:])
            nss_sb = cpool.tile([2 * W, NW], dt.float32, tag="nss")
            nc.sync.dma_start(nss_sb[:], nss_d[:])
            idx_sb = cpool.tile([128, T * 8], dt.int16, tag="idx")
            nc.sync.dma_start(idx_sb[:], idx_d[:])
            ldst_sb = cpool.tile([128, T], dt.bfloat16, tag="ldst")
            nc.sync.dma_start(ldst_sb[:], ldst_d[:])
            # x1acc: window w -> partitions (w%2)*64..+64, cols (w//2)*128..+128
            x1acc = cpool.tile([128, (NW + 1) // 2 * H], dt.float32, tag="acc")
            out_sb = cpool.tile([8, NW * W], dt.float32, tag="out")
            x1stage = cpool.tile([128, (NW + 1) // 2 * H], dt.bfloat16,
                                 tag="xst")

            ag_h_in = dram.tile([NSHP, H], dt.bfloat16)
            h_full = dram.tile([TROWS, H], dt.bfloat16, addr_space="Shared")
            ag_x_in = dram.tile([NSHP, H], dt.bfloat16)
            x_full = dram.tile([TROWS, H], dt.bfloat16, addr_space="Shared")

            # ---- stage B: h_sh = XnT_sh.T @ W1 ----
            for sb0 in range(0, KB, NSB):
                nsb = min(NSB, KB - sb0)
                xnt_sb = xpool.tile([128, KCH * NSB * 128], dt.bfloat16,
                                    tag="xnt")
                nc.sync.dma_start(
                    xnt_sb[:, :KCH * nsb * 128].rearrange(
                        "p (k n) -> p k n", k=KCH),
                    xnt_d[:, sb0 * 128:(sb0 + nsb) * 128].rearrange(
                        "(k p) n -> p k n", p=128))
                for nb in range(nsb):
                    ph = psA.tile([128, H], dt.float32, tag="ph")
                    for k in range(KCH):
                        nc.tensor.matmul(
                            out=ph[:],
                            lhsT=xnt_sb[:, (k * nsb + nb) * 128:
                                        (k * nsb + nb) * 128 + 128],
                            rhs=w1_sb[:, k * H:(k + 1) * H],
                            start=(k == 0), stop=(k == KCH - 1))
                    hb = spool.tile([128, H], dt.bfloat16, tag="hb")
                    nc.vector.tensor_copy(out=hb[:], in_=ph[:])
                    nc.sync.dma_start(
                        ag_h_in[(sb0 + nb) * 128:(sb0 + nb) * 128 + 128, :],
                        hb[:])

            if timing:
                nc.sync.dma_start(h_full[0:NSHP, :], ag_h_in[:])
            else:
                nc.gpsimd.collective_compute(
                    "AllGather", mybir.AluOpType.bypass,
                    replica_groups=[list(range(NCORES))],
                    ins=[ag_h_in[:].opt()], outs=[h_full[:].opt()])

            # ---- aggregation layers ----
            def aggregate(table, layer):
                nc.vector.memset(x1acc[:], 0.0)
                pw = {}      # window -> psum tile
                nmm = {}     # window -> matmuls issued this pass
                qn = [0]
                for hf in range(2):
                    t0 = 0 if hf == 0 else ntl
                    t1 = ntl if hf == 0 else T
                    tbl = table[0:LO, :] if hf == 0 else table[LO:TROWS, :]
                    for b0 in range(t0, t1, BT):
                        nt = min(BT, t1 - b0)
                        qn[0] = (qn[0] + 1) % NSWQ
                        gt = gpool.tile([128, BT * H], dt.bfloat16, tag="g")
                        nc.gpsimd.dma_gather(
                            out_ap=gt[:, :nt * H].rearrange(
                                "p (n e) -> p n e", e=H),
                            in_ap=tbl,
                            idxs_ap=idx_sb[:, b0 * 8:(b0 + nt) * 8],
                            num_idxs=nt * 128,
                            num_idxs_reg=nt * 128,
                            elem_size=H,
                            single_packet=False,
                            queue_num=qn[0])
                        oh = ohpool.tile([128, BT * W], dt.bfloat16, tag="oh")
                        ld = ldst_sb[:, b0:b0 + nt]
                        nc.vector.tensor_tensor(
                            out=oh[:, :nt * W].rearrange(
                                "p (n w) -> p n w", w=W),
                            in0=bass.AP(ld.tensor, ld.offset,
                                        ld.ap + [[0, W]]),
                            in1=bass.AP(iota_sb[:].tensor, iota_sb[:].offset,
                                        [iota_sb[:].ap[0], [0, nt],
                                         iota_sb[:].ap[1]]),
                            op=mybir.AluOpType.is_equal)
                        for j in range(nt):
                            t = b0 + j
                            w = tile_win[t]
                            po, co = (w % 2) * W, (w // 2) * H
                            if w not in pw:
                                pw[w] = psW.tile([2 * W, H], dt.float32,
                                                 name=f"pw{w}", tag="pw")
                                nmm[w] = 0
                                if hf == 1:  # seed from pass-lo partial
                                    nc.tensor.matmul(
                                        out=pw[w][po:po + W, :],
                                        lhsT=identf_sb[po:po + W, :],
                                        rhs=x1acc[po:po + W, co:co + H],
                                        start=True, stop=False)
                                    nmm[w] = 1
                            last_of_win = (t + 1 == t1 or
                                           tile_win[t + 1] != w)
                            nc.tensor.matmul(
                                out=pw[w][po:po + W, :],
                                lhsT=oh[:, j * W:(j + 1) * W],
                                rhs=gt[:, j * H:(j + 1) * H],
                                start=(nmm[w] == 0), stop=last_of_win)
                            nmm[w] += 1
                            if last_of_win:
                                if hf == 0:
                                    nc.vector.tensor_copy(
                                        out=x1acc[po:po + W, co:co + H],
                                        in_=pw[w][po:po + W, :])
                                    del pw[w]
                                else:
                                    win_epilogue(w, pw.pop(w), layer)
                assert not pw, f"unclosed windows {list(pw)}"
                # windows with zero hi tiles still need seed + epilogue
                for w in range(NW):
                    if tw[w, 1] == 0:
                        p = psW.tile([2 * W, H], dt.float32, tag="pw")
                        po, co = (w % 2) * W, (w // 2) * H
                        nc.tensor.matmul(
                            out=p[po:po + W, :], lhsT=identf_sb[po:po + W, :],
                            rhs=x1acc[po:po + W, co:co + H],
                            start=True, stop=True)
                        win_epilogue(w, p, layer)

            def win_epilogue(w, psum, layer):
                po = (w % 2) * W
                ps = psum[po:po + W, :]
                if layer == 0:
                    u = eppool.tile([2 * W, H], dt.float32, tag="ep")
                    uh = u[po:po + W, :]
                    nc.vector.tensor_scalar(
                        out=uh, in0=ps, scalar1=nsd_sb[po:po + W, w:w + 1],
                        scalar2=None, op0=mybir.AluOpType.mult)
                    nc.vector.tensor_tensor(
                        out=uh, in0=uh, in1=b1_sb[po:po + W, :],
                        op=mybir.AluOpType.add)
                    nc.vector.tensor_scalar(
                        out=uh, in0=uh, scalar1=nss_sb[po:po + W, w:w + 1],
                        scalar2=None, op0=mybir.AluOpType.mult)
                    co = (w // 2) * H
                    nc.vector.tensor_scalar(
                        out=x1stage[po:po + W, co:co + H], in0=uh,
                        scalar1=0.0, scalar2=None, op0=mybir.AluOpType.max)
                else:
                    vb = eppool.tile([2 * W, H], dt.bfloat16, tag="vb")
                    nc.vector.tensor_scalar(
                        out=vb[po:po + W, :], in0=ps,
                        scalar1=nsd_sb[po:po + W, w:w + 1],
                        scalar2=None, op0=mybir.AluOpType.mult)
                    pt = psT.tile([128, W], dt.bfloat16, tag="pt")
                    nc.tensor.transpose(out=pt[:], in_=vb[po:po + W, :],
                                        identity=identb_sb[po:po + W, :])
                    at = spool.tile([128, W], dt.bfloat16, tag="at")
                    nc.vector.tensor_copy(out=at[:], in_=pt[:])
                    p3 = psT.tile([8, W], dt.float32, tag="p3")
                    nc.tensor.matmul(out=p3[:], lhsT=w2_sb[:], rhs=at[:],
                                     start=True, stop=True)
                    nc.vector.tensor_copy(
                        out=out_sb[:, w * W:(w + 1) * W], in_=p3[:])

            if phases >= 1:
                aggregate(h_full, 0)
                for pr in range(NW // 2):
                    nc.sync.dma_start(
                        ag_x_in[pr * 128:(pr + 1) * 128, :],
                        x1stage[:, pr * H:(pr + 1) * H])
            if timing:
                nc.sync.dma_start(x_full[0:NSHP, :], ag_x_in[:])
            else:
                nc.gpsimd.collective_compute(
                    "AllGather", mybir.AluOpType.bypass,
                    replica_groups=[list(range(NCORES))],
                    ins=[ag_x_in[:].opt()], outs=[x_full[:].opt()])
            if phases >= 2:
                aggregate(x_full, 1)
            else:
                nc.vector.memset(out_sb[:], 0.0)

            nc.sync.dma_start(out_d[:], out_sb[:, :NSH])
            if DEBUG_TAPS:
                dbg_h = nc.dram_tensor("dbg_h", [TROWS, H], dt.bfloat16,
                                       kind="ExternalOutput")
                dbg_x = nc.dram_tensor("dbg_x", [TROWS, H], dt.bfloat16,
                                       kind="ExternalOutput")
                nc.sync.dma_start(dbg_h[:], h_full[:])
                nc.sync.dma_start(dbg_x[:], x_full[:])
    nc.compile()
    return nc


_CACHE = {}
_LAST_RES = None


def kernel(features, src, dst, W1, b1, W2, b2):
    in_maps, tw, tile_win, tile_base, T = _prep(
        features, src, dst, W1, b1, W2, b2)
    key = (T, tuple(tw.reshape(-1).tolist()))
    if key not in _CACHE:
        _CACHE[key] = _build_program(tw, tile_win, T)
    nc = _CACHE[key]
    res = bass_utils.run_bass_kernel_spmd(
        nc, in_maps, core_ids=list(range(NCORES)))
    global _LAST_RES
    _LAST_RES = res
    out = np.empty((N, C), np.float32)
    b2f = np.asarray(b2, np.float32)
    for c in range(NCORES):
        out[c * NSH:(c + 1) * NSH] = res.results[c]["out"].T[:, :C]
    out += b2f[None, :]
    return out



# revision 3
# speedup vs baseline: 1.0910x; 1.0910x over previous
"""2-layer GCN (DGL GraphConv norm='both') on 8 Trainium2 NeuronCores.

v2 strategy (dst-sharded layer 1, src-sharded scatter-add layer 2):
  - Host: degree norms; per-shard nodes PERMUTED by out-degree key
    max(d_lo, d_hi) so scatter layers cover block-prefixes of ranks;
    edges for L1 sorted by (dst-window, src-half, src) into 128-edge
    tiles; L2 edges colored into layers (one edge per src rank, distinct
    dst rows per layer -- dma_scatter_add races on same-row descriptors
    within one call, so each call must hit distinct rows).
  - Device per core c (shard = ranks of nodes [c*6250, (c+1)*6250)):
      h_sh = XnT_sh.T @ W1            (dense matmuls, bf16, f32 psum)
      AllGather h -> full h table [50176, 128] bf16 in HBM
      agg1 = segment_sum(h[src], dst) via dma_gather (rows->partitions)
             + onehot matmuls accumulating in PSUM per 64-dst window
      epilogue per window: x1n = relu((agg1*ndst + b1))*nsrc (bf16),
        transpose, x2win = x1n @ W2 -> x2stage [128, 49*8] f32
      scatter phase: for each (half, layer): dma_scatter_add 32B f32
        payloads of x2stage rank-prefix into HBM table [50176, 64] f32
        (256B row stride), garbage slots -> pad row 6271
      ReduceScatter(add) -> rs_out [6272, 64]; out = rs * ndst
  - Host: un-permute rows, add b2, concat cores.
"""

import numpy as np
import ml_dtypes

import concourse.bass as bass
import concourse.bacc as bacc
import concourse.mybir as mybir
import concourse.tile as tile
from concourse import bass_utils

BF16 = ml_dtypes.bfloat16

N = 50000
E = 1600000
FIN = 1433
FP = 1536            # FIN padded to 12*128
H = 128
C = 7
NCORES = 8
NSH = N // NCORES    # 6250
W = 64               # dst window width
NW = (NSH + W - 1) // W          # 98 windows per core
NSHP = NW * W                    # 6272 padded shard rows
TROWS = NCORES * NSHP            # 50176 table rows
LO = 32768                       # lo/hi h-table split (gather idx int16)
SPLIT = TROWS // 2               # 25088 scatter-table half split
BT = 48              # max tiles per dma_gather call
PAD_DST = 200.0      # local-dst sentinel for pad edges (> W-1)
GARBAGE = 6271       # scatter pad row (pad rank of core 0 / core 4)
STEP = 64            # scatter table row stride (f32) = 256B

KB = NW * W // 128   # 49 node blocks of 128 in dense stage
KCH = FP // 128      # 12 contraction chunks
GP_BUFS = 3          # gather-tile buffering
OH_BUFS = 2          # onehot-tile buffering
PSW_BUFS = 3         # window psum buffering
NSB = 8              # node blocks per xnt load in dense stage
NSWQ = 1             # SWDGE queues


def _ceil_div(a, b):
    return (a + b - 1) // b


def _wrap_idx(idx_flat):
    """[n] -> [128, n//16] int16: desc i -> [i%16 (+16k copies), i//16]."""
    a = np.asarray(idx_flat, np.int16).reshape(-1, 16).T
    return np.ascontiguousarray(np.tile(a, (8, 1)))


def _color_layers(rks, rows, nl_cap=500, seed=1):
    """Assign each edge (src rank, dst row) a layer: per layer, at most one
    edge per rank and all-distinct rows.  Ranks are degree-sorted, so rank
    r's edges go to layers 0..deg(r)-1 bijectively; iterative random swaps
    repair row conflicts; stubborn leftovers go to fresh layers."""
    rng = np.random.default_rng(seed)
    n = len(rks)
    o = np.argsort(rks, kind="stable")
    rks, rows = rks[o], rows[o]
    starts = np.searchsorted(rks, np.arange(NSH + 1))
    layer = np.arange(n) - starts[rks]
    for _ in range(300):
        key = layer.astype(np.int64) * (SPLIT + 1) + rows
        order = np.argsort(key, kind="stable")
        ks = key[order]
        dup = np.zeros(n, bool)
        dup[order[1:]] = ks[1:] == ks[:-1]
        if not dup.any():
            break
        idx = np.nonzero(dup)[0]
        rng.shuffle(idx)
        for i in idx:
            r = rks[i]
            j = rng.integers(starts[r], starts[r + 1])
            layer[i], layer[j] = layer[j], layer[i]
    else:
        # leftovers -> fresh layers; both rows AND ranks must stay unique
        # within each fresh layer (one idx-stream slot per rank per layer)
        key = layer.astype(np.int64) * (SPLIT + 1) + rows
        order = np.argsort(key, kind="stable")
        ks = key[order]
        dup = np.zeros(n, bool)
        dup[order[1:]] = ks[1:] == ks[:-1]
        nl = int(layer.max()) + 1
        used_rows = {}
        used_rks = {}
        for i in np.nonzero(dup)[0]:
            j = nl
            while (rows[i] in used_rows.setdefault(j, set())
                   or rks[i] in used_rks.setdefault(j, set())):
                j += 1
            used_rows[j].add(rows[i])
            used_rks[j].add(rks[i])
            layer[i] = j
    return rks, rows, layer


def _prep(features, src, dst, W1, b1, W2, b2):
    """Host-side sharding/packing. Returns (in_maps, program-shape params)."""
    src = np.asarray(src).astype(np.int64)
    dst = np.asarray(dst).astype(np.int64)
    features = np.asarray(features, np.float32)

    deg_src = np.bincount(src, minlength=N).astype(np.float32)
    deg_dst = np.bincount(dst, minlength=N).astype(np.float32)
    nsrc = 1.0 / np.sqrt(np.maximum(deg_src, 1.0))
    ndst = 1.0 / np.sqrt(np.maximum(deg_dst, 1.0))

    # ---- per-shard permutation by out-degree (lo/hi = dst core group) ----
    lo_edge = (dst // NSH) < (NCORES // 2)
    d_lo_g = np.bincount(src[lo_edge], minlength=N)
    d_hi_g = np.bincount(src[~lo_edge], minlength=N)
    perms = []
    rank_of_g = np.empty(N, np.int64)
    for c in range(NCORES):
        a, b = c * NSH, (c + 1) * NSH
        key = np.maximum(d_lo_g[a:b], d_hi_g[a:b])
        order = np.argsort(-key, kind="stable")     # rank -> orig local id
        perms.append(order)
        rank_of = np.empty(NSH, np.int64)
        rank_of[order] = np.arange(NSH)
        rank_of_g[a:b] = rank_of
    grank = (np.arange(N) // NSH) * NSHP + rank_of_g    # node -> table row

    g_src = grank[src]
    g_dst = grank[dst]

    # ---- L1 edge tiles (dst-sharded, window/half sorted) ----
    dcore = dst // NSH
    dloc = rank_of_g[dst]
    win = dloc // W
    half = (g_src >= LO).astype(np.int64)

    cnt = np.zeros((NCORES, NW, 2), np.int64)
    per_core = []
    for c in range(NCORES):
        m = dcore == c
        gs, wn, hf, dl = g_src[m], win[m], half[m], dloc[m]
        order = np.lexsort((gs, hf, wn))
        gs, wn, hf, dl = gs[order], wn[order], hf[order], dl[order]
        key = wn * 2 + hf
        cnt[c] = np.bincount(key, minlength=NW * 2).reshape(NW, 2)
        per_core.append((gs, dl, key))

    tw = np.zeros((NW, 2), np.int64)
    for h in range(2):
        tw[:, h] = _ceil_div(np.max(cnt[:, :, h], axis=0), 128)

    tile_win = []
    tile_base = np.zeros((NW, 2), np.int64)
    for h in range(2):
        for w in range(NW):
            tile_base[w, h] = len(tile_win)
            tile_win.extend([w] * int(tw[w, h]))
    T = len(tile_win)

    # ---- L2 scatter layers (src-sharded, collision-free per call) ----
    score = src // NSH
    s_half = (g_dst >= SPLIT).astype(np.int64)
    s_row = g_dst - s_half * SPLIT
    core_layers = []      # [c][h] -> (rks, rows, layer)
    nl = np.zeros((NCORES, 2), np.int64)
    for c in range(NCORES):
        m = score == c
        rk_c = rank_of_g[src[m]]
        hf_c = s_half[m]
        rw_c = s_row[m]
        res = []
        for h in (0, 1):
            mm = hf_c == h
            rks, rows, layer = _color_layers(rk_c[mm], rw_c[mm],
                                             seed=17 * c + h)
            res.append((rks, rows, layer))
            nl[c, h] = layer.max() + 1
        core_layers.append(res)

    # shared schedule: per (half, layer) block count = max over cores
    sched = []            # [(half, B)]
    for h in (0, 1):
        nlh = int(nl[:, h].max())
        for j in range(nlh):
            B = 0
            for c in range(NCORES):
                rks, rows, layer = core_layers[c][h]
                mj = layer == j
                if mj.any():
                    B = max(B, int(rks[mj].max()) // 128 + 1)
            sched.append((h, B))
    ndesc = sum(B * 128 for _, B in sched)

    # per-core scatter idx streams
    idx2s = []
    for c in range(NCORES):
        stream = np.empty(ndesc, np.int16)
        off = 0
        for h, B in sched:
            n = B * 128
            seg = np.full(n, GARBAGE, np.int16)
            rks, rows, layer = core_layers[c][h]
            # which layer does this sched entry correspond to?
            # sched is ordered: all lo layers then all hi layers
            stream[off:off + n] = seg
            off += n
        # fill real edges (second pass, tracking per-half layer offsets)
        off_of = {}
        off = 0
        lay_idx = {0: 0, 1: 0}
        for h, B in sched:
            off_of[(h, lay_idx[h])] = (off, B * 128)
            lay_idx[h] += 1
            off += B * 128
        for h in (0, 1):
            rks, rows, layer = core_layers[c][h]
            for j in range(int(nl[c, h])):
                mj = layer == j
                o0, n = off_of[(h, j)]
                stream[o0 + rks[mj]] = rows[mj].astype(np.int16)
        idx2s.append(stream)

    # ---- dense-stage feature prep ----
    Xn = features * nsrc[:, None]

    w1p = np.zeros((FP, H), np.float32)
    w1p[:FIN] = W1
    w1p = w1p.astype(BF16)
    w2p = np.zeros((H, 8), np.float32)
    w2p[:, :C] = W2
    w2p = w2p.astype(BF16)
    iota = np.tile(np.arange(W, dtype=np.float32), (128, 1)).astype(BF16)
    ident = np.vstack([np.eye(W, dtype=np.float32)] * 2)        # [128, 64]
    identb = ident.astype(BF16)
    b1rep = np.tile(np.asarray(b1, np.float32), (2 * W, 1))     # [128, 128]

    in_maps = []
    for c in range(NCORES):
        gs, dl, key = per_core[c]
        idx_flat = np.zeros(T * 128, np.int64)
        ldst_flat = np.full(T * 128, PAD_DST, np.float32)
        starts = np.zeros(NW * 2 + 1, np.int64)
        starts[1:] = np.cumsum(np.bincount(key, minlength=NW * 2))
        for h in range(2):
            for w in range(NW):
                k = w * 2 + h
                n = starts[k + 1] - starts[k]
                if n == 0:
                    continue
                slot = tile_base[w, h] * 128
                idx_flat[slot:slot + n] = gs[starts[k]:starts[k + 1]] - h * LO
                ldst_flat[slot:slot + n] = dl[starts[k]:starts[k + 1]] % W

        porder = perms[c]
        xnt = np.zeros((FP, NSHP), np.float32)
        xnt[:FIN, :NSH] = Xn[c * NSH + porder].T

        pad_d = np.zeros(NSHP, np.float32)
        pad_d[:NSH] = ndst[c * NSH + porder]
        nsd = pad_d.reshape(NW, W).T
        nsd = np.vstack([nsd, nsd])                             # [128, NW]
        pad_s = np.zeros(NSHP, np.float32)
        pad_s[:NSH] = nsrc[c * NSH + porder]
        nss = pad_s.reshape(NW, W).T
        nss = np.vstack([nss, nss])

        # final ndst multiplier in wrap layout [128, KB*8]
        v = pad_d.reshape(KB, 128).T                            # [128, 49]
        nfin8 = np.repeat(v[:, :, None], 8, axis=2).reshape(128, KB * 8)

        in_maps.append({
            "xnt": xnt.astype(BF16),
            "w1": w1p,
            "w2": w2p,
            "iota": iota,
            "identf": ident,
            "identb": identb,
            "b1rep": b1rep,
            "nsd": np.ascontiguousarray(nsd),
            "nss": np.ascontiguousarray(nss),
            "nfin8": np.ascontiguousarray(nfin8.astype(np.float32)),
            "idx": _wrap_idx(idx_flat),
            "idx2": _wrap_idx(idx2s[c]),
            "ldst": np.ascontiguousarray(
                ldst_flat.reshape(T, 128).T).astype(BF16),
        })
    return in_maps, tw, tile_win, tile_base, T, sched, ndesc, perms


def _build_program(tw, tile_win, T, sched, ndesc, timing=False):
    nc = bacc.Bacc("TRN2", target_bir_lowering=False, debug=False,
                   num_devices=NCORES, num_swdge_queues=NSWQ)
    dt = mybir.dt
    xnt_d = nc.dram_tensor("xnt", [FP, NSHP], dt.bfloat16, kind="ExternalInput")
    w1_d = nc.dram_tensor("w1", [FP, H], dt.bfloat16, kind="ExternalInput")
    w2_d = nc.dram_tensor("w2", [H, 8], dt.bfloat16, kind="ExternalInput")
    iota_d = nc.dram_tensor("iota", [128, W], dt.bfloat16, kind="ExternalInput")
    identf_d = nc.dram_tensor("identf", [2 * W, W], dt.float32, kind="ExternalInput")
    identb_d = nc.dram_tensor("identb", [2 * W, W], dt.bfloat16, kind="ExternalInput")
    b1_d = nc.dram_tensor("b1rep", [2 * W, H], dt.float32, kind="ExternalInput")
    nsd_d = nc.dram_tensor("nsd", [2 * W, NW], dt.float32, kind="ExternalInput")
    nss_d = nc.dram_tensor("nss", [2 * W, NW], dt.float32, kind="ExternalInput")
    nfin_d = nc.dram_tensor("nfin8", [128, KB * 8], dt.float32, kind="ExternalInput")
    idx_d = nc.dram_tensor("idx", [128, T * 8], dt.int16, kind="ExternalInput")
    idx2_d = nc.dram_tensor("idx2", [128, ndesc // 16], dt.int16,
                            kind="ExternalInput")
    ldst_d = nc.dram_tensor("ldst", [128, T], dt.bfloat16, kind="ExternalInput")
    out_d = nc.dram_tensor("out", [128, KB * 8], dt.float32, kind="ExternalOutput")

    ntl = int(tw[:, 0].sum())

    with tile.TileContext(nc) as tc:
        with (
            tc.tile_pool(name="const", bufs=1) as cpool,
            tc.tile_pool(name="xnt", bufs=2) as xpool,
            tc.tile_pool(name="g", bufs=GP_BUFS) as gpool,
            tc.tile_pool(name="oh", bufs=OH_BUFS) as ohpool,
            tc.tile_pool(name="ep", bufs=2) as eppool,
            tc.tile_pool(name="small", bufs=2) as spool,
            tc.tile_pool(name="psA", bufs=2, space="PSUM") as psA,
            tc.tile_pool(name="psW", bufs=PSW_BUFS, space="PSUM") as psW,
            tc.tile_pool(name="psT", bufs=1, space="PSUM") as psT,
            tc.tile_pool(name="ps3", bufs=2, space="PSUM") as ps3,
            tc.tile_pool(name="dram", bufs=1, space="DRAM") as dram,
        ):
            # ---- constants ----
            w1_sb = cpool.tile([128, KCH * H], dt.bfloat16, tag="w1")
            nc.sync.dma_start(
                w1_sb[:].rearrange("p (k h) -> p k h", h=H),
                w1_d[:].rearrange("(k p) h -> p k h", p=128))
            w2_sb = cpool.tile([128, 8], dt.bfloat16, tag="w2")
            nc.sync.dma_start(w2_sb[:], w2_d[:])
            iota_sb = cpool.tile([128, W], dt.bfloat16, tag="iota")
            nc.sync.dma_start(iota_sb[:], iota_d[:])
            identf_sb = cpool.tile([2 * W, W], dt.float32, tag="idf")
            nc.sync.dma_start(identf_sb[:], identf_d[:])
            identb_sb = cpool.tile([2 * W, W], dt.bfloat16, tag="idb")
            nc.sync.dma_start(identb_sb[:], identb_d[:])
            b1_sb = cpool.tile([2 * W, H], dt.float32, tag="b1")
            nc.sync.dma_start(b1_sb[:], b1_d[:])
            nsd_sb = cpool.tile([2 * W, NW], dt.float32, tag="nsd")
            nc.sync.dma_start(nsd_sb[:], nsd_d[:])
            nss_sb = cpool.tile([2 * W, NW], dt.float32, tag="nss")
            nc.sync.dma_start(nss_sb[:], nss_d[:])
            nfin_sb = cpool.tile([128, KB * 8], dt.float32, tag="nfin")
            nc.sync.dma_start(nfin_sb[:], nfin_d[:])
            idx_sb = cpool.tile([128, T * 8], dt.int16, tag="idx")
            nc.sync.dma_start(idx_sb[:], idx_d[:])
            idx2_sb = cpool.tile([128, ndesc // 16], dt.int16, tag="idx2")
            nc.sync.dma_start(idx2_sb[:], idx2_d[:])
            ldst_sb = cpool.tile([128, T], dt.bfloat16, tag="ldst")
            nc.sync.dma_start(ldst_sb[:], ldst_d[:])
            # x1acc: window w -> partitions (w%2)*64..+64, cols (w//2)*128..+128
            x1acc = cpool.tile([128, (NW + 1) // 2 * H], dt.float32, tag="acc")
            x2stage = cpool.tile([128, KB * 8], dt.float32, tag="x2st")
            zr = cpool.tile([128, KB * 8], dt.float32, tag="zr")
            nc.vector.memset(zr[:], 0.0)

            ag_h_in = dram.tile([NSHP, H], dt.bfloat16)
            h_full = dram.tile([TROWS, H], dt.bfloat16, addr_space="Shared")
            tbl = dram.tile([TROWS, STEP], dt.float32)
            rs_out = dram.tile([NSHP, STEP], dt.float32)

            # zero the scatter table payload columns (cols 0:8 of each row)
            for k in range(NCORES):
                nc.sync.dma_start(
                    tbl[k * NSHP:(k + 1) * NSHP, 0:8].rearrange(
                        "(b p) f -> p b f", p=128),
                    zr[:].rearrange("p (b f) -> p b f", f=8))

            # ---- stage B: h_sh = XnT_sh.T @ W1 ----
            for sb0 in range(0, KB, NSB):
                nsb = min(NSB, KB - sb0)
                xnt_sb = xpool.tile([128, KCH * NSB * 128], dt.bfloat16,
                                    tag="xnt")
                nc.sync.dma_start(
                    xnt_sb[:, :KCH * nsb * 128].rearrange(
                        "p (k n) -> p k n", k=KCH),
                    xnt_d[:, sb0 * 128:(sb0 + nsb) * 128].rearrange(
                        "(k p) n -> p k n", p=128))
                for nb in range(nsb):
                    ph = psA.tile([128, H], dt.float32, tag="ph")
                    for k in range(KCH):
                        nc.tensor.matmul(
                            out=ph[:],
                            lhsT=xnt_sb[:, (k * nsb + nb) * 128:
                                        (k * nsb + nb) * 128 + 128],
                            rhs=w1_sb[:, k * H:(k + 1) * H],
                            start=(k == 0), stop=(k == KCH - 1))
                    hb = spool.tile([128, H], dt.bfloat16, tag="hb")
                    nc.vector.tensor_copy(out=hb[:], in_=ph[:])
                    nc.sync.dma_start(
                        ag_h_in[(sb0 + nb) * 128:(sb0 + nb) * 128 + 128, :],
                        hb[:])

            if timing:
                nc.sync.dma_start(h_full[0:NSHP, :], ag_h_in[:])
            else:
                nc.gpsimd.collective_compute(
                    "AllGather", mybir.AluOpType.bypass,
                    replica_groups=[list(range(NCORES))],
                    ins=[ag_h_in[:].opt()], outs=[h_full[:].opt()])

            # ---- layer-1 aggregation over dst windows ----
            def win_epilogue(w, psum):
                po = (w % 2) * W
                co8 = (w // 2) * 8
                ps = psum[po:po + W, :]
                u = eppool.tile([2 * W, H], dt.float32, tag="ep")
                uh = u[po:po + W, :]
                nc.vector.tensor_scalar(
                    out=uh, in0=ps, scalar1=nsd_sb[po:po + W, w:w + 1],
                    scalar2=None, op0=mybir.AluOpType.mult)
                nc.vector.tensor_tensor(
                    out=uh, in0=uh, in1=b1_sb[po:po + W, :],
                    op=mybir.AluOpType.add)
                vb = eppool.tile([2 * W, H], dt.bfloat16, tag="vb")
                nc.vector.tensor_scalar(
                    out=vb[po:po + W, :], in0=uh,
                    scalar1=nss_sb[po:po + W, w:w + 1],
                    scalar2=0.0, op0=mybir.AluOpType.mult,
                    op1=mybir.AluOpType.max)
                pt = psT.tile([128, W], dt.bfloat16, tag="pt")
                nc.tensor.transpose(out=pt[:], in_=vb[po:po + W, :],
                                    identity=identb_sb[po:po + W, :])
                at = spool.tile([128, W], dt.bfloat16, tag="at")
                nc.vector.tensor_copy(out=at[:], in_=pt[:])
                p3 = ps3.tile([2 * W, 8], dt.float32, tag="p3")
                nc.tensor.matmul(out=p3[po:po + W, :], lhsT=at[:],
                                 rhs=w2_sb[:], start=True, stop=True)
                nc.vector.tensor_copy(
                    out=x2stage[po:po + W, co8:co8 + 8],
                    in_=p3[po:po + W, :])

            nc.vector.memset(x1acc[:], 0.0)
            pw = {}      # window -> psum tile
            nmm = {}     # window -> matmuls issued this pass
            for hf in range(2):
                t0 = 0 if hf == 0 else ntl
                t1 = ntl if hf == 0 else T
                tbl_h = h_full[0:LO, :] if hf == 0 else h_full[LO:TROWS, :]
                for b0 in range(t0, t1, BT):
                    nt = min(BT, t1 - b0)
                    gt = gpool.tile([128, BT * H], dt.bfloat16, tag="g")
                    nc.gpsimd.dma_gather(
                        out_ap=gt[:, :nt * H].rearrange(
                            "p (n e) -> p n e", e=H),
                        in_ap=tbl_h,
                        idxs_ap=idx_sb[:, b0 * 8:(b0 + nt) * 8],
                        num_idxs=nt * 128,
                        num_idxs_reg=nt * 128,
                        elem_size=H,
                        single_packet=False,
                        queue_num=0)
                    oh = ohpool.tile([128, BT * W], dt.bfloat16, tag="oh")
                    ld = ldst_sb[:, b0:b0 + nt]
                    nc.vector.tensor_tensor(
                        out=oh[:, :nt * W].rearrange(
                            "p (n w) -> p n w", w=W),
                        in0=bass.AP(ld.tensor, ld.offset,
                                    ld.ap + [[0, W]]),
                        in1=bass.AP(iota_sb[:].tensor, iota_sb[:].offset,
                                    [iota_sb[:].ap[0], [0, nt],
                                     iota_sb[:].ap[1]]),
                        op=mybir.AluOpType.is_equal)
                    for j in range(nt):
                        t = b0 + j
                        w = tile_win[t]
                        po, co = (w % 2) * W, (w // 2) * H
                        if w not in pw:
                            pw[w] = psW.tile([2 * W, H], dt.float32,
                                             name=f"pw{w}", tag="pw")
                            nmm[w] = 0
                            if hf == 1:  # seed from pass-lo partial
                                nc.tensor.matmul(
                                    out=pw[w][po:po + W, :],
                                    lhsT=identf_sb[po:po + W, :],
                                    rhs=x1acc[po:po + W, co:co + H],
                                    start=True, stop=False)
                                nmm[w] = 1
                        last_of_win = (t + 1 == t1 or
                                       tile_win[t + 1] != w)
                        nc.tensor.matmul(
                            out=pw[w][po:po + W, :],
                            lhsT=oh[:, j * W:(j + 1) * W],
                            rhs=gt[:, j * H:(j + 1) * H],
                            start=(nmm[w] == 0), stop=last_of_win)
                        nmm[w] += 1
                        if last_of_win:
                            if hf == 0:
                                nc.vector.tensor_copy(
                                    out=x1acc[po:po + W, co:co + H],
                                    in_=pw[w][po:po + W, :])
                                del pw[w]
                            else:
                                win_epilogue(w, pw.pop(w))
            assert not pw, f"unclosed windows {list(pw)}"
            # windows with zero hi tiles still need seed + epilogue
            for w in range(NW):
                if tw[w, 1] == 0:
                    p = psW.tile([2 * W, H], dt.float32, tag="pw")
                    po, co = (w % 2) * W, (w // 2) * H
                    nc.tensor.matmul(
                        out=p[po:po + W, :], lhsT=identf_sb[po:po + W, :],
                        rhs=x1acc[po:po + W, co:co + H],
                        start=True, stop=True)
                    win_epilogue(w, p)

            # ---- layer-2: scatter-add x2 rows into the dst table ----
            off16 = 0
            for h, B in sched:
                n = B * 128
                out_view = (tbl[0:SPLIT, 0:8] if h == 0
                            else tbl[SPLIT:TROWS, 0:8])
                nc.gpsimd.dma_scatter_add(
                    out_view,
                    x2stage[:, 0:B * 8].rearrange("p (n e) -> p n e", e=8),
                    idx2_sb[:, off16:off16 + n // 16],
                    n, n, 8,
                    elem_step=STEP,
                    queue_num=0)
                off16 += n // 16

            if timing:
                nc.sync.dma_start(rs_out[:], tbl[0:NSHP, :])
            else:
                nc.gpsimd.collective_compute(
                    "ReduceScatter", mybir.AluOpType.add,
                    replica_groups=[list(range(NCORES))],
                    ins=[tbl[:].opt()], outs=[rs_out[:].opt()])

            # ---- finish: out = rs * ndst  (wrap layout [128, 49*8]) ----
            fin = cpool.tile([128, KB * 8], dt.float32, tag="fin")
            nc.sync.dma_start(
                fin[:].rearrange("p (b f) -> p b f", f=8),
                rs_out[:, 0:8].rearrange("(b p) f -> p b f", p=128))
            nc.vector.tensor_tensor(
                out=fin[:], in0=fin[:], in1=nfin_sb[:],
                op=mybir.AluOpType.mult)
            nc.sync.dma_start(out_d[:], fin[:])
    nc.compile()
    return nc


_CACHE = {}
_LAST_RES = None


def kernel(features, src, dst, W1, b1, W2, b2):
    in_maps, tw, tile_win, tile_base, T, sched, ndesc, perms = _prep(
        features, src, dst, W1, b1, W2, b2)
    key = (T, tuple(tw.reshape(-1).tolist()), tuple(sched))
    if key not in _CACHE:
        _CACHE[key] = _build_program(tw, tile_win, T, sched, ndesc)
    nc = _CACHE[key]
    res = bass_utils.run_bass_kernel_spmd(
        nc, in_maps, core_ids=list(range(NCORES)))
    global _LAST_RES
    _LAST_RES = res
    out = np.empty((N, C), np.float32)
    b2f = np.asarray(b2, np.float32)
    for c in range(NCORES):
        arr = res.results[c]["out"].reshape(128, KB, 8)
        by_rank = arr.transpose(1, 0, 2).reshape(NSHP, 8)
        out[c * NSH + perms[c]] = by_rank[:NSH, :C]
    out += b2f[None, :]
    return out
